# revision 7
# baseline (speedup 1.0000x reference)
"""Trainium2 Bass kernel for segment-softmax graph attention pooling.

Computation (see reference):
    proj = h @ a                                  # (M, D)
    s[i] = x[i] . proj[seg[i]]                    # per-node score
    att  = segment_softmax(s)                     # softmax within each segment
    out[g] = sum_{i in seg g} att[i] * x[i]       # (M, D)

Sharding: 512 graphs (and their contiguous nodes -- segment_ids is sorted)
per core. Inside a core, graphs are grouped into 16 windows of W=32 graphs.
The host pads each window's nodes to a uniform tile budget T_w (the global
max) so the tile -> window mapping is a compile-time constant shared by all
8 cores (single SPMD NEFF). Scores skip the segment-max subtraction: scores
are tiny (|s| < ~1), so exp() is safe and softmax is algebraically identical.

All tensors travel as fp16; accumulation stays in f32 PSUM. Per chunk of
16 128-node tiles:
  1. xT per tile via PE transpose (fp16, 8 tiles per PSUM bank, one
     psum->sbuf copy per 8)
  2. s_all[i, 0:32] = xT.T @ projT[:, window] per tile on PE
  3. ea = exp(s_all) on ScalarE (fp16 out); es = ea * sel on DVE, where
     sel is a host-built one-hot of each node's graph within the window
  4. psum[window, 0:129] += es.T @ [x | 1]  -> col 0:128 = unnormalized
     output, col 128 = softmax denominator z. Finalize: out = psum/(z+eps),
     batched 4 windows per output DMA.
The output matmuls for chunk ci are issued AFTER the scores of chunk ci+2
(two-chunk software pipeline) so the PE never stalls waiting for exp.
"""

import numpy as np

import concourse.bacc as bacc
import concourse.bass as bass
import concourse.tile as tile
from concourse import mybir
from concourse.bass_utils import run_bass_kernel_spmd
from concourse.masks import make_identity

N_CORES = 8
M = 4096          # graphs
N = 262144        # nodes
D = 128           # feature dim
GPC = M // N_CORES        # graphs per core = 512
W = 32                    # graphs per window
WPC = GPC // W            # windows per core = 16
C = 16                    # tiles per chunk

F32 = mybir.dt.float32
FP16 = mybir.dt.float16


def _build_program(T_w: int, n_chunks: int):
    """Build + compile the SPMD program for a per-window tile budget T_w."""
    T_pad = n_chunks * C

    def win_of(t):
        return min(t // T_w, WPC - 1)

    def win_first(w):
        return w * T_w

    def win_last(w):
        return (w + 1) * T_w - 1 if w < WPC - 1 else T_pad - 1

    nc = bacc.Bacc("TRN2", target_bir_lowering=False, debug=False,
                   num_devices=N_CORES)

    h_d = nc.dram_tensor("h", [GPC, D], FP16, kind="ExternalInput")
    a_d = nc.dram_tensor("a", [D, D], FP16, kind="ExternalInput")
    xe_d = nc.dram_tensor("xe", [128, T_pad, D + 1], FP16, kind="ExternalInput")
    sel_d = nc.dram_tensor("sel", [128, T_pad, W], FP16, kind="ExternalInput")
    out_d = nc.dram_tensor("out", [GPC, D], F32, kind="ExternalOutput")

    with tile.TileContext(nc) as tc:
        with (
            tc.tile_pool(name="const", bufs=1) as const_pool,
            tc.tile_pool(name="xc", bufs=7) as x_pool,
            tc.tile_pool(name="selc", bufs=5) as sel_pool,
            tc.tile_pool(name="xt", bufs=3) as xt_pool,
            tc.tile_pool(name="ework", bufs=5) as ea_pool,
            tc.tile_pool(name="eswork", bufs=5) as es_pool,
            tc.tile_pool(name="fin", bufs=3) as fin_pool,
            tc.tile_pool(name="ps_xt", bufs=2, space="PSUM") as psum_xt,
            tc.tile_pool(name="ps_s", bufs=2, space="PSUM") as psum_s,
            tc.tile_pool(name="ps_o", bufs=1, space="PSUM") as psum_o,
        ):
            xe_v = xe_d.ap()   # [128, T_pad, D+1], per-partition contiguous
            sel_v = sel_d.ap()

            def load_chunk(ci, split=False):
                """DMA chunk ci: x alternates between the two hwdge queues
                (sync/scalar) to double delivery bandwidth; sel on gpsimd.

                split=True halves the x DMA across BOTH queues so the first
                chunk reaches the PE sooner at startup."""
                xc = x_pool.tile([128, C, D + 1], FP16, tag="xc",
                                 name=f"xc{ci}" if ci < 2 else "xc")
                if split:
                    h0 = C // 2
                    nc.sync.dma_start(xc[:, 0:h0, :],
                                      xe_v[:, ci * C:ci * C + h0, :])
                    nc.scalar.dma_start(xc[:, h0:C, :],
                                        xe_v[:, ci * C + h0:(ci + 1) * C, :])
                else:
                    eng = nc.sync if ci % 2 == 0 else nc.scalar
                    eng.dma_start(xc[:], xe_v[:, ci * C:(ci + 1) * C, :])
                sc = sel_pool.tile([128, C, W], FP16, tag="sc",
                                   name=f"sc{ci}" if ci < 2 else "sc")
                nc.gpsimd.dma_start(sc[:], sel_v[:, ci * C:(ci + 1) * C, :])
                return xc, sc

            # prefetch first chunks (split across queues so compute starts
            # early)
            pre = [load_chunk(0, split=True), load_chunk(1),
                   load_chunk(2)]

            # ---- preamble: identity, a, projT = (h @ a).T ----
            ident_h = const_pool.tile([128, 128], FP16)
            make_identity(nc, ident_h[:])

            a_sb = const_pool.tile([128, D], FP16)
            nc.gpsimd.dma_start(a_sb[:], a_d.ap())

            h4 = const_pool.tile([128, 4, D], FP16)
            nc.gpsimd.dma_start(h4[:], h_d.ap().rearrange("(t p) k -> p t k", p=128))
            # hT[k, g] = h[g, k] via the XBAR (batched per-128 transpose;
            # one-off, so the hwdge occupancy doesn't matter here)
            hT = const_pool.tile([128, 4, 128], FP16)
            nc.scalar.dma_start(hT[:], h4[:], transpose=True)

            p_pt = psum_s.tile([128, 512], F32, tag="ps", name="p_pt")
            # projT[j, g] = sum_k a[k, j] * hT[k, g]
            nc.tensor.matmul(p_pt[:], a_sb[:], hT[:], start=True, stop=True)
            projT = const_pool.tile([128, GPC], FP16)
            nc.vector.tensor_copy(projT[:], p_pt[:])

            # ---- output accumulators: 2 banks x [128, 129] (4 windows/bank)
            po = [psum_o.tile([128, D + 1], F32, tag=f"bank{b}",
                              name=f"po_bank{b}")
                  for b in range(2)]

            ob_group = {}

            def emit_outputs(ci, xc, es):
                """Output matmuls + window finalize for chunk ci (skewed)."""
                for t in range(C):
                    g = ci * C + t
                    w = win_of(g)
                    b = (w // 4) % 2
                    poff = 32 * (w % 4)
                    # psum[gw, 0:129] += sum_i es[i, gw] * [x | 1][i, :]
                    nc.tensor.matmul(po[b][poff:poff + W, :],
                                     es[:, t, :], xc[:, t, :],
                                     start=(g == win_first(w)),
                                     stop=(g == win_last(w)),
                                     tile_position=(0, poff))
                    if g == win_last(w):
                        # finalize window w: out = acc / (z + eps)
                        sl = slice(poff, poff + W)
                        zt = fin_pool.tile([128, 1], F32, tag="z", name="zt")
                        nc.vector.tensor_scalar_add(zt[sl, :],
                                                    po[b][sl, D:D + 1], 1e-30)
                        rz = fin_pool.tile([128, 1], F32, tag="rz", name="rz")
                        nc.vector.reciprocal(rz[sl, :], zt[sl, :])
                        if w % 4 == 0:
                            ob_group[w // 4] = fin_pool.tile(
                                [128, D], F32, tag="ob", name="ob")
                        ob = ob_group[w // 4]
                        nc.vector.tensor_scalar_mul(ob[sl, :], po[b][sl, 0:D],
                                                    rz[sl, :])
                        if w % 4 == 3:
                            # flush 4 windows = 128 output rows in one DMA
                            w0 = w - 3
                            nc.gpsimd.dma_start(
                                out_d.ap()[w0 * W:w0 * W + 128, :], ob[:])

            pending = []
            # ---- main loop ----
            for ci in range(n_chunks):
                xc, sc = pre[ci] if ci < 3 else loaded.pop(0)
                if ci == 0:
                    loaded = []
                if ci + 3 < n_chunks:
                    loaded.append(load_chunk(ci + 3))

                ps = psum_s.tile([128, C, W], F32, tag="ps", name="ps")
                xts_q = []
                for q in range(C // 8):
                    pxt = psum_xt.tile([128, 1024], FP16, tag="pxt", name="pxt")
                    for k in range(8):
                        t = q * 8 + k
                        # xT tile via PE transpose mode (fp16)
                        nc.tensor.transpose(pxt[:, k * 128:(k + 1) * 128],
                                            xc[:, t, 0:D], ident_h[:])
                    xts = xt_pool.tile([128, 1024], FP16)
                    # one psum->sbuf copy per 8 tiles; alternate engines
                    if q == 0:
                        nc.vector.tensor_copy(xts[:], pxt[:])
                    else:
                        nc.scalar.copy(xts[:], pxt[:])
                    xts_q.append(xts)
                # scores after ALL transposes: the psum->sbuf copies get
                # latency cover from the second transpose batch
                for q in range(C // 8):
                    for k in range(8):
                        t = q * 8 + k
                        w = win_of(ci * C + t)
                        # s_all[i, gw] = sum_j xT[j, i] * projT[j, 32w + gw]
                        nc.tensor.matmul(ps[:, t, :],
                                         xts_q[q][:, k * 128:(k + 1) * 128],
                                         projT[:, w * W:(w + 1) * W],
                                         start=True, stop=True)

                ea = ea_pool.tile([128, C, W], FP16, tag="ea")
                nc.scalar.activation(ea[:], ps[:],
                                     mybir.ActivationFunctionType.Exp)
                es = es_pool.tile([128, C, W], FP16, tag="es")
                nc.vector.tensor_mul(es[:], ea[:], sc[:])

                # two-chunk skew: output matmuls trail the score pipeline
                pending.append((ci, xc, es))
                if len(pending) > 2:
                    emit_outputs(*pending.pop(0))

            for p in pending:
                emit_outputs(*p)

    nc.compile()
    return nc


def _prep_inputs(h, x, a, segment_ids):
    """Shard + window-pad inputs; returns (T_w, n_chunks, in_maps)."""
    seg = np.ascontiguousarray(segment_ids).astype(np.int64)
    x = np.ascontiguousarray(x, dtype=np.float32)
    h = np.ascontiguousarray(h, dtype=np.float16)
    a = np.ascontiguousarray(a, dtype=np.float16)

    counts = np.bincount(seg, minlength=M)
    win_nodes = counts.reshape(M // W, W).sum(axis=1)          # [128]
    win_starts = np.concatenate([[0], np.cumsum(win_nodes)])[:-1]
    T_w = max(1, int(np.ceil(win_nodes.max() / 128)))
    T = WPC * T_w
    n_chunks = (T + C - 1) // C
    T_pad = n_chunks * C

    in_maps = []
    for c in range(N_CORES):
        xe = np.zeros((T_pad * 128, D + 1), dtype=np.float16)
        xe[:, D] = 1.0
        sel = np.zeros((T_pad * 128, W), dtype=np.float16)
        for w in range(WPC):
            wg = c * WPC + w
            s0 = int(win_starts[wg])
            n = int(win_nodes[wg])
            if n == 0:
                continue
            r0 = w * T_w * 128
            xe[r0:r0 + n, 0:D] = x[s0:s0 + n]
            lg = (seg[s0:s0 + n] - wg * W).astype(np.int64)
            sel[r0 + np.arange(n), lg] = 1.0
        in_maps.append({
            "h": h[c * GPC:(c + 1) * GPC],
            "a": a,
            "xe": np.ascontiguousarray(
                xe.reshape(T_pad, 128, D + 1).transpose(1, 0, 2)),
            "sel": np.ascontiguousarray(
                sel.reshape(T_pad, 128, W).transpose(1, 0, 2)),
        })
    return T_w, n_chunks, in_maps


_prog_cache = {}


def _get_program(T_w, n_chunks):
    key = (T_w, n_chunks)
    if key not in _prog_cache:
        _prog_cache[key] = _build_program(T_w, n_chunks)
    return _prog_cache[key]


def kernel(h, x, a, segment_ids, _trace=False):
    assert h.shape == (M, D) and x.shape == (N, D) and a.shape == (D, D)
    T_w, n_chunks, in_maps = _prep_inputs(h, x, a, segment_ids)
    nc = _get_program(T_w, n_chunks)
    res = run_bass_kernel_spmd(nc, in_maps, core_ids=list(range(N_CORES)),
                               trace=_trace)
    out = np.concatenate([res.results[c]["out"] for c in range(N_CORES)], axis=0)
    if _trace:
        kernel.last_result = res
    return out
